# revision 35
# baseline (speedup 1.0000x reference)
"""
Multi-head attention (B=2, T=2048, D=1024, H=16, causal) on 8 TRN2 NeuronCores.

Sharding: batch x head-group. Core c handles batch b=c//4 and the 4 heads
h in [4*(c%4), 4*(c%4)+4). Projection weights are column-sharded (Wq/Wk/Wv
rows, Wo cols); each core computes a partial output projection over its own
256 head-channels and the host sums the 4 partials per batch.

Host-side prep (numpy, outside the NEFF): transpose + bf16-cast of inputs and
weight shards so the device needs zero on-chip transposes:
  - qt/kt/vt:  X[b].T        -> [1024, 2048] bf16 (contraction dim on partitions)
  - wqt/wkt/wvt: Wx[rows,:].T -> [1024, 256] bf16
  - wot:       Wo[:, cols].T -> [256, 1024] bf16

Device dataflow (all matmuls bf16, fp32 PSUM accumulation), pipelined per
512-wide t-block so the ACT-paced attention of block i overlaps the dense
projection matmuls of block i+1 (keeps the PE queue deep -> warm HAM clock,
hidden weight loads):
  qT/kT [256,2048] and v [2048,256] projections ->
  per (t-block, head-pair): scoresT [s,t] blocks (causal-trimmed), ONE exp on
  ACT per pair (scale=1/8 folded in), attn@v accumulates avT [65,512] in PSUM
  where row 64 is the softmax denominator (ones-column trick) ->
  f32 denominator broadcast on GpSimd + approx-reciprocal + DVE mul ->
  normalized oT [256,2048] -> output projection -> partial out [2048,1024]
  f32 -> HBM.
"""

import os
import sys

import numpy as np

sys.path.insert(0, "/opt/trn_rl_repo")

import ml_dtypes  # noqa: E402

import concourse.bass as bass  # noqa: E402
from concourse import bacc  # noqa: E402
import concourse.mybir as mybir  # noqa: E402
from concourse.tile import TileContext  # noqa: E402

B, T, D = 2, 2048, 1024
H, DK = 16, 64
HL = 4  # heads per core
DL = HL * DK  # 256 local head-channels per core
NCORES = 8
KC = D // 128  # 8 contraction chunks of 128
TCN = T // 128  # 16 t-chunks
TBN = T // 512  # 4 t-blocks

BF = mybir.dt.bfloat16
F32 = mybir.dt.float32
_BF_NP = ml_dtypes.bfloat16


def build_nc(mode: str, debug: bool = False) -> bass.Bass:
    """mode: 'causal' | 'full' | 'generic' (generic = arbitrary 0/1 mask)."""
    from contextlib import ExitStack

    assert mode in ("causal", "full", "generic")
    causal = mode == "causal"

    nc = bacc.Bacc()
    if debug:
        dbg_qT = nc.declare_dram_parameter("dbg_qT", [128, 2, T], BF, isOutput=True)
        dbg_kT = nc.declare_dram_parameter("dbg_kT", [128, 2, T], BF, isOutput=True)
        dbg_v = nc.declare_dram_parameter(
            "dbg_v", [128, TCN, HL, 65], BF, isOutput=True
        )
        dbg_oT = nc.declare_dram_parameter("dbg_oT", [128, 2, T], BF, isOutput=True)
    qt_d = nc.declare_dram_parameter("qt", [TBN, 128, KC, 512], BF, isOutput=False)
    kt_d = nc.declare_dram_parameter("kt", [TBN, 128, KC, 512], BF, isOutput=False)
    vt_d = nc.declare_dram_parameter("vt", [TBN, 128, KC, 512], BF, isOutput=False)
    wqt_d = nc.declare_dram_parameter("wqt", [128, KC, DL], BF, isOutput=False)
    wkt_d = nc.declare_dram_parameter("wkt", [128, KC, DL], BF, isOutput=False)
    wvt_d = nc.declare_dram_parameter("wvt", [128, KC, DL], BF, isOutput=False)
    wot_d = nc.declare_dram_parameter("wot", [128, 2, D], BF, isOutput=False)
    if causal:
        dtril_d = nc.declare_dram_parameter("dtril", [128, 128], BF, isOutput=False)
    if mode == "generic":
        # maskT[sc, p, t] = mask[t, sc*128+p] as 0/1 bf16 (multiplied post-exp)
        maskt_d = nc.declare_dram_parameter(
            "maskt", [128, TCN, T], BF, isOutput=False
        )
    out_d = nc.declare_dram_parameter("out", [T, D], F32, isOutput=True)

    with TileContext(nc) as tc, ExitStack() as ctx:
        const = ctx.enter_context(tc.tile_pool(name="const", bufs=1))
        sexp = ctx.enter_context(tc.tile_pool(name="sexp", bufs=6))
        souts = ctx.enter_context(tc.tile_pool(name="souts", bufs=2))
        snorm = ctx.enter_context(tc.tile_pool(name="snorm", bufs=2))

        # ---- resident SBUF tensors -------------------------------------
        qt_sb = const.tile([128, KC, T], BF, tag="qt_sb")
        kt_sb = const.tile([128, KC, T], BF, tag="kt_sb")
        vt_sb = const.tile([128, KC, T], BF, tag="vt_sb")
        wq_sb = const.tile([128, KC, DL], BF, tag="wq_sb")
        wk_sb = const.tile([128, KC, DL], BF, tag="wk_sb")
        wv_sb = const.tile([128, KC, DL], BF, tag="wv_sb")
        wo_sb = const.tile([128, 2, D], BF, tag="wo_sb")
        qT_sb = const.tile([128, 2, T], BF, tag="qT_sb")  # q transposed
        kT_sb = const.tile([128, 2, T], BF, tag="kT_sb")
        v_sb = const.tile([128, TCN, HL, 65], BF, tag="v_sb")  # v + ones col
        oT_sb = const.tile([128, 2, T], BF, tag="oT_sb")
        if causal:
            dtril_sb = const.tile([128, 128], BF, tag="dtril_sb")

        # inputs arrive in per-t-block chunks so projections can start after
        # the first ~3MB instead of waiting for all 12MB; alternate between
        # the two HWDGE rings (sync/scalar) to parallelize issue
        nc.sync.dma_start(out=wq_sb, in_=wqt_d[:])
        nc.sync.dma_start(out=wk_sb, in_=wkt_d[:])
        nc.sync.dma_start(out=wv_sb, in_=wvt_d[:])
        nc.sync.dma_start(out=wo_sb, in_=wot_d[:])
        if causal:
            nc.sync.dma_start(out=dtril_sb, in_=dtril_d[:])
        for tb in range(TBN):
            for sb, dr in ((qt_sb, qt_d), (kt_sb, kt_d), (vt_sb, vt_d)):
                nc.sync.dma_start(
                    out=sb[:, :, tb * 512 : (tb + 1) * 512], in_=dr[tb]
                )
        nc.any.memset(v_sb[:, :, :, 64], 1.0)  # denominator ones column

        pproj = ctx.enter_context(tc.tile_pool(name="pproj", bufs=2, space="PSUM"))
        pscore = ctx.enter_context(
            tc.tile_pool(name="pscore", bufs=2, space="PSUM")
        )
        pav = ctx.enter_context(tc.tile_pool(name="pav", bufs=2, space="PSUM"))

        def proj_qkT(y_sb, w_sb, x_sb, tb):
            # qT/kT [dh, t]: lhsT = WxT chunk [128,128], rhs = XT chunk [128,512]
            for mc in range(2):
                ps = pproj.tile([128, 512], F32, tag="pp", name="ps")
                for kc in range(KC):
                    nc.tensor.matmul(
                        ps,
                        w_sb[:, kc, mc * 128 : (mc + 1) * 128],
                        x_sb[:, kc, tb * 512 : (tb + 1) * 512],
                        start=(kc == 0),
                        stop=(kc == KC - 1),
                    )
                nc.vector.tensor_copy(y_sb[:, mc, tb * 512 : (tb + 1) * 512], ps)

        def proj_v(tcki):
            # v natural [t, dh]: lhsT = VT chunk [128,128], rhs = WvT [128,256]
            psf = pproj.tile([128, 512], F32, tag="pp", name="psf")
            ps = psf[:, 0:256]
            for kc in range(KC):
                nc.tensor.matmul(
                    ps,
                    vt_sb[:, kc, tcki * 128 : (tcki + 1) * 128],
                    wv_sb[:, kc, :],
                    start=(kc == 0),
                    stop=(kc == KC - 1),
                )
            # scatter 4 heads of 64 into the 65-strided v_sb (col 64 = ones)
            nc.vector.tensor_copy(
                v_sb[:, tcki, :, 0:64],
                ps.rearrange("p (h d) -> p h d", h=HL),
            )

        def attention_pair(tb, p):
            # heads 2p (partitions 0:64) and 2p+1 (64:128) of chunk ch=p.
            # One 2-bank PSUM score tile and ONE exp instruction serve both
            # heads (halves the ACT instruction count and sync traffic).
            scn = 4 * tb + 4 if causal else TCN
            av0 = pav.tile([65, 512], F32, tag="av", name="av0")
            av1 = pav.tile([65, 512], F32, tag="av", name="av1")
            avs = (av0, av1)
            for sc in range(scn):
                c0 = max(0, 128 * sc - 512 * tb) if causal else 0
                diag = causal and sc >= 4 * tb
                sp = pscore.tile([128, 2, 512], F32, tag="sp", name="sp")
                for i in range(2):
                    po = i * 64
                    nc.tensor.matmul(
                        sp[:, i, c0:512],
                        kT_sb[po : po + 64, p, sc * 128 : (sc + 1) * 128],
                        qT_sb[po : po + 64, p, tb * 512 + c0 : (tb + 1) * 512],
                        start=True,
                        stop=True,
                    )
                # diag tiles use a separate ex pool so the tril-mul on them
                # never stalls later exps recycling plain tiles
                ex = sexp.tile(
                    [128, 2, 512],
                    BF,
                    tag="exd" if diag else "ex",
                    name="exd" if diag else "ex",
                )
                nc.scalar.activation(
                    ex[:, :, c0:512],
                    sp[:, :, c0:512],
                    mybir.ActivationFunctionType.Exp,
                    scale=0.125,
                )
                if diag:
                    # diagonal 128-col block: zero out s > t via tril mask
                    for i in range(2):
                        nc.vector.tensor_mul(
                            ex[:, i, c0 : c0 + 128],
                            ex[:, i, c0 : c0 + 128],
                            dtril_sb,
                        )
                if mode == "generic":
                    mt = sexp.tile([128, 512], BF, tag="mt", name="mt")
                    nc.sync.dma_start(
                        out=mt, in_=maskt_d[:, sc, tb * 512 : (tb + 1) * 512]
                    )
                    for i in range(2):
                        nc.vector.tensor_mul(ex[:, i, :], ex[:, i, :], mt)
                for i in range(2):
                    nc.tensor.matmul(
                        avs[i][:, c0:512],
                        v_sb[:, sc, 2 * p + i, :],
                        ex[:, i, c0:512],
                        start=(sc == 0),
                        stop=(sc == scn - 1),
                    )
            for i in range(2):
                po = i * 64
                av = avs[i]
                # normalize: oT = av[0:64] / broadcast(av[64]). The f32
                # denominator row is partition-broadcast on GpSimd, then
                # approx-reciprocal across 64 DVE lanes (a [1,512] exact
                # reciprocal runs on one lane and costs ~3.4us).
                dn32 = snorm.tile([1, 512], F32, tag="dn32", name="dn32")
                nc.vector.tensor_copy(dn32, av[64:65, :])
                dnb = snorm.tile([64, 512], F32, tag="dnb", name="dnb")
                nc.gpsimd.partition_broadcast(dnb, dn32)
                rc = snorm.tile([64, 512], F32, tag="rc", name="rc")
                nc.vector.reciprocal_approx_fast(rc, dnb)
                nc.vector.tensor_mul(
                    oT_sb[po : po + 64, p, tb * 512 : (tb + 1) * 512],
                    av[0:64, :],
                    rc,
                )

        def out_proj(tcki):
            ob = souts.tile([128, D], F32, tag="ob", name="ob")
            pof = pscore.tile([128, 2, 512], F32, tag="sp", name="pof")
            for nj in range(2):
                for c in range(2):
                    nc.tensor.matmul(
                        pof[:, nj, :],
                        oT_sb[:, c, tcki * 128 : (tcki + 1) * 128],
                        wo_sb[:, c, nj * 512 : (nj + 1) * 512],
                        start=(c == 0),
                        stop=(c == 1),
                    )
            for nj in range(2):
                nc.vector.tensor_copy(
                    ob[:, nj * 512 : (nj + 1) * 512], pof[:, nj, :]
                )
            nc.sync.dma_start(out=out_d[tcki * 128 : (tcki + 1) * 128, :], in_=ob)

        if causal:
            # t-block 0's projections up front; block tb+1's projections are
            # emitted BETWEEN block tb's two attention pairs so the PE filler
            # spreads evenly through each ACT-paced stretch and the next
            # block's attention starts without a projection gap.
            proj_qkT(qT_sb, wq_sb, qt_sb, 0)
            proj_qkT(kT_sb, wk_sb, kt_sb, 0)
            for tcki in range(4):
                proj_v(tcki)
            for tb in range(TBN):
                attention_pair(tb, 0)
                if tb + 1 < TBN:
                    proj_qkT(qT_sb, wq_sb, qt_sb, tb + 1)
                    proj_qkT(kT_sb, wk_sb, kt_sb, tb + 1)
                attention_pair(tb, 1)
                if tb + 1 < TBN:
                    for tcki in range(4 * tb + 4, 4 * tb + 8):
                        proj_v(tcki)
                for tcki in range(4 * tb, 4 * tb + 4):
                    out_proj(tcki)
        else:
            for tb in range(TBN):
                if tb == 0:
                    # non-causal needs the full kT/v before any attention
                    for tbp in range(TBN):
                        proj_qkT(qT_sb, wq_sb, qt_sb, tbp)
                        proj_qkT(kT_sb, wk_sb, kt_sb, tbp)
                        for tcki in range(4 * tbp, 4 * tbp + 4):
                            proj_v(tcki)
                for p in range(2):
                    attention_pair(tb, p)
                for tcki in range(4 * tb, 4 * tb + 4):
                    out_proj(tcki)

        if debug:
            nc.sync.dma_start(out=dbg_qT[:], in_=qT_sb)
            nc.sync.dma_start(out=dbg_kT[:], in_=kT_sb)
            nc.sync.dma_start(out=dbg_v[:], in_=v_sb)
            nc.sync.dma_start(out=dbg_oT[:], in_=oT_sb)

    nc.finalize()
    return nc


_NC_CACHE: dict = {}


def _get_nc(mode: str) -> bass.Bass:
    if mode not in _NC_CACHE:
        _NC_CACHE[mode] = build_nc(mode)
    return _NC_CACHE[mode]


def _mask_mode(mask2d: np.ndarray) -> str:
    n = mask2d.shape[0]
    if np.array_equal(mask2d != 0, np.tril(np.ones((n, n), dtype=bool))):
        return "causal"
    if np.all(mask2d != 0):
        return "full"
    return "generic"


def make_in_maps(Q, K, V, Wq, Wk, Wv, Wo, mask):
    mask2d = np.asarray(mask)[0, 0]
    mode = _mask_mode(mask2d)

    def sh_in(x):  # [T, D] f32 -> transposed bf16, tb-chunked [TBN, 128, KC, 512]
        xt = np.ascontiguousarray(x.T).astype(_BF_NP).reshape(KC, 128, TBN, 512)
        return np.ascontiguousarray(xt.transpose(2, 1, 0, 3))

    def sh_w(w):  # [DL, D] rows -> WxT [D, DL] bf16 partition-major [128, KC, DL]
        wt = np.ascontiguousarray(w.T).astype(_BF_NP).reshape(KC, 128, DL)
        return np.ascontiguousarray(wt.transpose(1, 0, 2))

    if mode == "causal":
        i = np.arange(128)
        # tril in the scoresT orientation: keep (s', t') where t' >= s'
        dtril = np.where(i[None, :] >= i[:, None], 1.0, 0.0).astype(_BF_NP)
    if mode == "generic":
        maskt = np.ascontiguousarray(
            np.ascontiguousarray((mask2d != 0).T)
            .astype(_BF_NP)
            .reshape(TCN, 128, T)
            .transpose(1, 0, 2)
        )

    qts = [sh_in(np.asarray(Q)[b]) for b in range(B)]
    kts = [sh_in(np.asarray(K)[b]) for b in range(B)]
    vts = [sh_in(np.asarray(V)[b]) for b in range(B)]

    in_maps = []
    for c in range(NCORES):
        b, hg = c // 4, c % 4
        rows = slice(hg * DL, (hg + 1) * DL)
        im = {
            "qt": qts[b],
            "kt": kts[b],
            "vt": vts[b],
            "wqt": sh_w(np.asarray(Wq)[rows, :]),
            "wkt": sh_w(np.asarray(Wk)[rows, :]),
            "wvt": sh_w(np.asarray(Wv)[rows, :]),
            "wot": np.ascontiguousarray(
                np.ascontiguousarray(np.asarray(Wo)[:, rows].T)
                .astype(_BF_NP)
                .reshape(2, 128, D)
                .transpose(1, 0, 2)
            ),
        }
        if mode == "causal":
            im["dtril"] = dtril
        if mode == "generic":
            im["maskt"] = maskt
        in_maps.append(im)
    return mode, in_maps


def _ensure_ntff_hook():
    """Install the antenv.axon_hooks shim so trace=True works under axon.

    The agent image's antenv package lacks axon_hooks; synthesize it and
    register the ctypes-based NTFF profile hook from trn_agent_boot.
    """
    import types

    try:
        from antenv import axon_hooks  # noqa: F401

        return True
    except ImportError:
        pass
    try:
        import antenv
        from trn_agent_boot.trn_boot import _ntff_profile_via_ctypes

        hook = _ntff_profile_via_ctypes("/opt/axon/libaxon_pjrt.so")
        mod = types.ModuleType("antenv.axon_hooks")
        _state = {"hook": hook}
        mod.get_axon_ntff_profile_hook = lambda: _state["hook"]

        def _set(h):
            _state["hook"] = h

        mod.set_axon_ntff_profile_hook = _set
        sys.modules["antenv.axon_hooks"] = mod
        antenv.axon_hooks = mod
        return hook is not None
    except Exception as e:  # degrade to no-trace
        print(f"ntff hook shim failed: {type(e).__name__}: {e}", file=sys.stderr)
        return False


def kernel(Q, K, V, Wq, Wk, Wv, Wo, mask):
    from concourse.bass_utils import run_bass_kernel_spmd

    mode, in_maps = make_in_maps(Q, K, V, Wq, Wk, Wv, Wo, mask)
    nc = _get_nc(mode)
    trace = bool(int(os.environ.get("KERNEL_TRACE", "0")))
    if trace and not _ensure_ntff_hook():
        trace = False
    res = run_bass_kernel_spmd(nc, in_maps, list(range(NCORES)), trace=trace)
    if trace:
        kernel.last_exec_time_ns = res.exec_time_ns
        kernel.last_results = res
    out = np.zeros((B, T, D), dtype=np.float32)
    for c in range(NCORES):
        out[c // 4] += res.results[c]["out"].astype(np.float32)
    return out


kernel.last_exec_time_ns = None
kernel.last_results = None


# revision 39
# speedup vs baseline: 1.0070x; 1.0070x over previous
"""
Multi-head attention (B=2, T=2048, D=1024, H=16, causal) on 8 TRN2 NeuronCores.

Sharding: batch x head-group. Core c handles batch b=c//4 and the 4 heads
h in [4*(c%4), 4*(c%4)+4). Projection weights are column-sharded (Wq/Wk/Wv
rows, Wo cols); each core computes a partial output projection over its own
256 head-channels and the host sums the 4 partials per batch.

Host-side prep (numpy, outside the NEFF): transpose + bf16-cast of inputs and
weight shards so the device needs zero on-chip transposes:
  - qt/kt/vt:  X[b].T        -> [1024, 2048] bf16 (contraction dim on partitions)
  - wqt/wkt/wvt: Wx[rows,:].T -> [1024, 256] bf16
  - wot:       Wo[:, cols].T -> [256, 1024] bf16

Device dataflow (all matmuls bf16, fp32 PSUM accumulation), pipelined per
512-wide t-block so the ACT-paced attention of block i overlaps the dense
projection matmuls of block i+1 (keeps the PE queue deep -> warm HAM clock,
hidden weight loads):
  qT/kT [256,2048] and v [2048,256] projections ->
  per (t-block, head-pair): scoresT [s,t] blocks (causal-trimmed), ONE exp on
  ACT per pair (scale=1/8 folded in), attn@v accumulates avT [65,512] in PSUM
  where row 64 is the softmax denominator (ones-column trick) ->
  f32 denominator broadcast on GpSimd + approx-reciprocal + DVE mul ->
  normalized oT [256,2048] -> output projection -> partial out [2048,1024]
  f32 -> HBM.
"""

import os
import sys

import numpy as np

sys.path.insert(0, "/opt/trn_rl_repo")

import ml_dtypes  # noqa: E402

import concourse.bass as bass  # noqa: E402
from concourse import bacc  # noqa: E402
import concourse.mybir as mybir  # noqa: E402
from concourse.tile import TileContext  # noqa: E402

B, T, D = 2, 2048, 1024
H, DK = 16, 64
HL = 4  # heads per core
DL = HL * DK  # 256 local head-channels per core
NCORES = 8
KC = D // 128  # 8 contraction chunks of 128
TCN = T // 128  # 16 t-chunks
TBN = T // 512  # 4 t-blocks

BF = mybir.dt.bfloat16
F32 = mybir.dt.float32
_BF_NP = ml_dtypes.bfloat16


def build_nc(mode: str, debug: bool = False) -> bass.Bass:
    """mode: 'causal' | 'full' | 'generic' (generic = arbitrary 0/1 mask)."""
    from contextlib import ExitStack

    assert mode in ("causal", "full", "generic")
    causal = mode == "causal"

    nc = bacc.Bacc()
    if debug:
        dbg_qT = nc.declare_dram_parameter("dbg_qT", [128, 2, T], BF, isOutput=True)
        dbg_kT = nc.declare_dram_parameter("dbg_kT", [128, 2, T], BF, isOutput=True)
        dbg_v = nc.declare_dram_parameter(
            "dbg_v", [128, TCN, HL, 80], BF, isOutput=True
        )
        dbg_oT = nc.declare_dram_parameter("dbg_oT", [128, 2, T], BF, isOutput=True)
    qt_d = nc.declare_dram_parameter("qt", [TBN, 128, KC, 512], BF, isOutput=False)
    kt_d = nc.declare_dram_parameter("kt", [TBN, 128, KC, 512], BF, isOutput=False)
    vt_d = nc.declare_dram_parameter("vt", [TBN, 128, KC, 512], BF, isOutput=False)
    wqt_d = nc.declare_dram_parameter("wqt", [128, KC, DL], BF, isOutput=False)
    wkt_d = nc.declare_dram_parameter("wkt", [128, KC, DL], BF, isOutput=False)
    wvt_d = nc.declare_dram_parameter("wvt", [128, KC, DL], BF, isOutput=False)
    wot_d = nc.declare_dram_parameter("wot", [128, 2, D], BF, isOutput=False)
    if causal:
        dtril_d = nc.declare_dram_parameter("dtril", [128, 128], BF, isOutput=False)
    if mode == "generic":
        # maskT[sc, p, t] = mask[t, sc*128+p] as 0/1 bf16 (multiplied post-exp)
        maskt_d = nc.declare_dram_parameter(
            "maskt", [128, TCN, T], BF, isOutput=False
        )
    out_d = nc.declare_dram_parameter("out", [T, D], F32, isOutput=True)

    with TileContext(nc) as tc, ExitStack() as ctx:
        const = ctx.enter_context(tc.tile_pool(name="const", bufs=1))
        sexp = ctx.enter_context(tc.tile_pool(name="sexp", bufs=6))
        souts = ctx.enter_context(tc.tile_pool(name="souts", bufs=2))
        snorm = ctx.enter_context(tc.tile_pool(name="snorm", bufs=2))

        # ---- resident SBUF tensors -------------------------------------
        qt_sb = const.tile([128, KC, T], BF, tag="qt_sb")
        kt_sb = const.tile([128, KC, T], BF, tag="kt_sb")
        vt_sb = const.tile([128, KC, T], BF, tag="vt_sb")
        wq_sb = const.tile([128, KC, DL], BF, tag="wq_sb")
        wk_sb = const.tile([128, KC, DL], BF, tag="wk_sb")
        wv_sb = const.tile([128, KC, DL], BF, tag="wv_sb")
        wo_sb = const.tile([128, 2, D], BF, tag="wo_sb")
        qT_sb = const.tile([128, 2, T], BF, tag="qT_sb")  # q transposed
        kT_sb = const.tile([128, 2, T], BF, tag="kT_sb")
        v_sb = const.tile([128, TCN, HL, 80], BF, tag="v_sb")  # v + ones col, padded to 160B stride
        oT_sb = const.tile([128, 2, T], BF, tag="oT_sb")
        if causal:
            dtril_sb = const.tile([128, 128], BF, tag="dtril_sb")

        # inputs arrive in per-t-block chunks so projections can start after
        # the first ~3MB instead of waiting for all 12MB; alternate between
        # the two HWDGE rings (sync/scalar) to parallelize issue
        nc.sync.dma_start(out=wq_sb, in_=wqt_d[:])
        nc.sync.dma_start(out=wk_sb, in_=wkt_d[:])
        nc.sync.dma_start(out=wv_sb, in_=wvt_d[:])
        nc.sync.dma_start(out=wo_sb, in_=wot_d[:])
        if causal:
            nc.sync.dma_start(out=dtril_sb, in_=dtril_d[:])
        for tb in range(TBN):
            for sb, dr in ((qt_sb, qt_d), (kt_sb, kt_d), (vt_sb, vt_d)):
                nc.sync.dma_start(
                    out=sb[:, :, tb * 512 : (tb + 1) * 512], in_=dr[tb]
                )
        nc.any.memset(v_sb[:, :, :, 64:80], 1.0)  # ones col (+pad init)

        pproj = ctx.enter_context(tc.tile_pool(name="pproj", bufs=2, space="PSUM"))
        pscore = ctx.enter_context(
            tc.tile_pool(name="pscore", bufs=2, space="PSUM")
        )
        pav = ctx.enter_context(tc.tile_pool(name="pav", bufs=2, space="PSUM"))

        def proj_qkT(y_sb, w_sb, x_sb, tb):
            # qT/kT [dh, t]: lhsT = WxT chunk [128,128], rhs = XT chunk [128,512]
            for mc in range(2):
                ps = pproj.tile([128, 512], F32, tag="pp", name="ps")
                for kc in range(KC):
                    nc.tensor.matmul(
                        ps,
                        w_sb[:, kc, mc * 128 : (mc + 1) * 128],
                        x_sb[:, kc, tb * 512 : (tb + 1) * 512],
                        start=(kc == 0),
                        stop=(kc == KC - 1),
                    )
                nc.vector.tensor_copy(y_sb[:, mc, tb * 512 : (tb + 1) * 512], ps)

        def proj_v(tcki):
            # v natural [t, dh]: lhsT = VT chunk [128,128], rhs = WvT [128,256]
            psf = pproj.tile([128, 512], F32, tag="pp", name="psf")
            ps = psf[:, 0:256]
            for kc in range(KC):
                nc.tensor.matmul(
                    ps,
                    vt_sb[:, kc, tcki * 128 : (tcki + 1) * 128],
                    wv_sb[:, kc, :],
                    start=(kc == 0),
                    stop=(kc == KC - 1),
                )
            # scatter 4 heads of 64 into the 65-strided v_sb (col 64 = ones)
            nc.vector.tensor_copy(
                v_sb[:, tcki, :, 0:64],
                ps.rearrange("p (h d) -> p h d", h=HL),
            )

        def attention_pair(tb, p):
            # heads 2p (partitions 0:64) and 2p+1 (64:128) of chunk ch=p.
            # One 2-bank PSUM score tile and ONE exp instruction serve both
            # heads (halves the ACT instruction count and sync traffic).
            scn = 4 * tb + 4 if causal else TCN
            av0 = pav.tile([65, 512], F32, tag="av", name="av0")
            av1 = pav.tile([65, 512], F32, tag="av", name="av1")
            avs = (av0, av1)
            for sc in range(scn):
                c0 = max(0, 128 * sc - 512 * tb) if causal else 0
                diag = causal and sc >= 4 * tb
                sp = pscore.tile([128, 2, 512], F32, tag="sp", name="sp")
                for i in range(2):
                    po = i * 64
                    nc.tensor.matmul(
                        sp[:, i, c0:512],
                        kT_sb[po : po + 64, p, sc * 128 : (sc + 1) * 128],
                        qT_sb[po : po + 64, p, tb * 512 + c0 : (tb + 1) * 512],
                        start=True,
                        stop=True,
                    )
                # diag tiles use a separate ex pool so the tril-mul on them
                # never stalls later exps recycling plain tiles
                ex = sexp.tile(
                    [128, 2, 512],
                    BF,
                    tag="exd" if diag else "ex",
                    name="exd" if diag else "ex",
                )
                nc.scalar.activation(
                    ex[:, :, c0:512],
                    sp[:, :, c0:512],
                    mybir.ActivationFunctionType.Exp,
                    scale=0.125,
                )
                if diag:
                    # diagonal 128-col block: zero out s > t via tril mask
                    for i in range(2):
                        nc.vector.tensor_mul(
                            ex[:, i, c0 : c0 + 128],
                            ex[:, i, c0 : c0 + 128],
                            dtril_sb,
                        )
                if mode == "generic":
                    mt = sexp.tile([128, 512], BF, tag="mt", name="mt")
                    nc.sync.dma_start(
                        out=mt, in_=maskt_d[:, sc, tb * 512 : (tb + 1) * 512]
                    )
                    for i in range(2):
                        nc.vector.tensor_mul(ex[:, i, :], ex[:, i, :], mt)
                for i in range(2):
                    nc.tensor.matmul(
                        avs[i][:, c0:512],
                        v_sb[:, sc, 2 * p + i, 0:65],
                        ex[:, i, c0:512],
                        start=(sc == 0),
                        stop=(sc == scn - 1),
                    )
            for i in range(2):
                po = i * 64
                av = avs[i]
                # normalize: oT = av[0:64] / broadcast(av[64]). The f32
                # denominator row is partition-broadcast on GpSimd, then
                # approx-reciprocal across 64 DVE lanes (a [1,512] exact
                # reciprocal runs on one lane and costs ~3.4us).
                dn32 = snorm.tile([1, 512], F32, tag="dn32", name="dn32")
                nc.vector.tensor_copy(dn32, av[64:65, :])
                dnb = snorm.tile([64, 512], F32, tag="dnb", name="dnb")
                nc.gpsimd.partition_broadcast(dnb, dn32)
                rc = snorm.tile([64, 512], F32, tag="rc", name="rc")
                nc.vector.reciprocal_approx_fast(rc, dnb)
                nc.vector.tensor_mul(
                    oT_sb[po : po + 64, p, tb * 512 : (tb + 1) * 512],
                    av[0:64, :],
                    rc,
                )

        def out_proj(tcki):
            ob = souts.tile([128, D], F32, tag="ob", name="ob")
            pof = pscore.tile([128, 2, 512], F32, tag="sp", name="pof")
            for nj in range(2):
                for c in range(2):
                    nc.tensor.matmul(
                        pof[:, nj, :],
                        oT_sb[:, c, tcki * 128 : (tcki + 1) * 128],
                        wo_sb[:, c, nj * 512 : (nj + 1) * 512],
                        start=(c == 0),
                        stop=(c == 1),
                    )
            for nj in range(2):
                nc.vector.tensor_copy(
                    ob[:, nj * 512 : (nj + 1) * 512], pof[:, nj, :]
                )
            nc.sync.dma_start(out=out_d[tcki * 128 : (tcki + 1) * 128, :], in_=ob)

        if causal:
            # t-block 0's projections up front; block tb+1's projections are
            # emitted BETWEEN block tb's two attention pairs so the PE filler
            # spreads evenly through each ACT-paced stretch and the next
            # block's attention starts without a projection gap.
            proj_qkT(qT_sb, wq_sb, qt_sb, 0)
            proj_qkT(kT_sb, wk_sb, kt_sb, 0)
            for tcki in range(4):
                proj_v(tcki)
            for tb in range(TBN):
                attention_pair(tb, 0)
                if tb + 1 < TBN:
                    proj_qkT(qT_sb, wq_sb, qt_sb, tb + 1)
                    proj_qkT(kT_sb, wk_sb, kt_sb, tb + 1)
                attention_pair(tb, 1)
                if tb + 1 < TBN:
                    for tcki in range(4 * tb + 4, 4 * tb + 8):
                        proj_v(tcki)
                for tcki in range(4 * tb, 4 * tb + 4):
                    out_proj(tcki)
        else:
            for tb in range(TBN):
                if tb == 0:
                    # non-causal needs the full kT/v before any attention
                    for tbp in range(TBN):
                        proj_qkT(qT_sb, wq_sb, qt_sb, tbp)
                        proj_qkT(kT_sb, wk_sb, kt_sb, tbp)
                        for tcki in range(4 * tbp, 4 * tbp + 4):
                            proj_v(tcki)
                for p in range(2):
                    attention_pair(tb, p)
                for tcki in range(4 * tb, 4 * tb + 4):
                    out_proj(tcki)

        if debug:
            nc.sync.dma_start(out=dbg_qT[:], in_=qT_sb)
            nc.sync.dma_start(out=dbg_kT[:], in_=kT_sb)
            nc.sync.dma_start(out=dbg_v[:], in_=v_sb)
            nc.sync.dma_start(out=dbg_oT[:], in_=oT_sb)

    nc.finalize()
    return nc


_NC_CACHE: dict = {}


def _get_nc(mode: str) -> bass.Bass:
    if mode not in _NC_CACHE:
        _NC_CACHE[mode] = build_nc(mode)
    return _NC_CACHE[mode]


def _mask_mode(mask2d: np.ndarray) -> str:
    n = mask2d.shape[0]
    if np.array_equal(mask2d != 0, np.tril(np.ones((n, n), dtype=bool))):
        return "causal"
    if np.all(mask2d != 0):
        return "full"
    return "generic"


def make_in_maps(Q, K, V, Wq, Wk, Wv, Wo, mask):
    mask2d = np.asarray(mask)[0, 0]
    mode = _mask_mode(mask2d)

    def sh_in(x):  # [T, D] f32 -> transposed bf16, tb-chunked [TBN, 128, KC, 512]
        xt = np.ascontiguousarray(x.T).astype(_BF_NP).reshape(KC, 128, TBN, 512)
        return np.ascontiguousarray(xt.transpose(2, 1, 0, 3))

    def sh_w(w):  # [DL, D] rows -> WxT [D, DL] bf16 partition-major [128, KC, DL]
        wt = np.ascontiguousarray(w.T).astype(_BF_NP).reshape(KC, 128, DL)
        return np.ascontiguousarray(wt.transpose(1, 0, 2))

    if mode == "causal":
        i = np.arange(128)
        # tril in the scoresT orientation: keep (s', t') where t' >= s'
        dtril = np.where(i[None, :] >= i[:, None], 1.0, 0.0).astype(_BF_NP)
    if mode == "generic":
        maskt = np.ascontiguousarray(
            np.ascontiguousarray((mask2d != 0).T)
            .astype(_BF_NP)
            .reshape(TCN, 128, T)
            .transpose(1, 0, 2)
        )

    qts = [sh_in(np.asarray(Q)[b]) for b in range(B)]
    kts = [sh_in(np.asarray(K)[b]) for b in range(B)]
    vts = [sh_in(np.asarray(V)[b]) for b in range(B)]

    in_maps = []
    for c in range(NCORES):
        b, hg = c // 4, c % 4
        rows = slice(hg * DL, (hg + 1) * DL)
        im = {
            "qt": qts[b],
            "kt": kts[b],
            "vt": vts[b],
            "wqt": sh_w(np.asarray(Wq)[rows, :]),
            "wkt": sh_w(np.asarray(Wk)[rows, :]),
            "wvt": sh_w(np.asarray(Wv)[rows, :]),
            "wot": np.ascontiguousarray(
                np.ascontiguousarray(np.asarray(Wo)[:, rows].T)
                .astype(_BF_NP)
                .reshape(2, 128, D)
                .transpose(1, 0, 2)
            ),
        }
        if mode == "causal":
            im["dtril"] = dtril
        if mode == "generic":
            im["maskt"] = maskt
        in_maps.append(im)
    return mode, in_maps


def _ensure_ntff_hook():
    """Install the antenv.axon_hooks shim so trace=True works under axon.

    The agent image's antenv package lacks axon_hooks; synthesize it and
    register the ctypes-based NTFF profile hook from trn_agent_boot.
    """
    import types

    try:
        from antenv import axon_hooks  # noqa: F401

        return True
    except ImportError:
        pass
    try:
        import antenv
        from trn_agent_boot.trn_boot import _ntff_profile_via_ctypes

        hook = _ntff_profile_via_ctypes("/opt/axon/libaxon_pjrt.so")
        mod = types.ModuleType("antenv.axon_hooks")
        _state = {"hook": hook}
        mod.get_axon_ntff_profile_hook = lambda: _state["hook"]

        def _set(h):
            _state["hook"] = h

        mod.set_axon_ntff_profile_hook = _set
        sys.modules["antenv.axon_hooks"] = mod
        antenv.axon_hooks = mod
        return hook is not None
    except Exception as e:  # degrade to no-trace
        print(f"ntff hook shim failed: {type(e).__name__}: {e}", file=sys.stderr)
        return False


def kernel(Q, K, V, Wq, Wk, Wv, Wo, mask):
    from concourse.bass_utils import run_bass_kernel_spmd

    mode, in_maps = make_in_maps(Q, K, V, Wq, Wk, Wv, Wo, mask)
    nc = _get_nc(mode)
    trace = bool(int(os.environ.get("KERNEL_TRACE", "0")))
    if trace and not _ensure_ntff_hook():
        trace = False
    res = run_bass_kernel_spmd(nc, in_maps, list(range(NCORES)), trace=trace)
    if trace:
        kernel.last_exec_time_ns = res.exec_time_ns
        kernel.last_results = res
    out = np.zeros((B, T, D), dtype=np.float32)
    for c in range(NCORES):
        out[c // 4] += res.results[c]["out"].astype(np.float32)
    return out


kernel.last_exec_time_ns = None
kernel.last_results = None
